# revision 32
# baseline (speedup 1.0000x reference)
"""Multi-head attention layer on 8 Trainium2 NeuronCores.

Problem: B=4, S=2048, D=1024, H=16 heads (hd=64), fp32 in/out.

Sharding: core c -> (batch b = c//2, head-group g = c%2). Each core computes
8 heads of one batch element. Fully data/tensor-parallel; no collectives.

Per-core dataflow (bf16 matmuls, fp32 PSUM accumulation):
  - host ships x[b].T (D on partitions) and W[g-slice].T, cast to bf16;
    bq/bk/bv packed into ONE [128, 264] f32 piece (tiny per-partition DMAs
    cost ~45ns/descriptor if shipped separately).
  - qT = (x @ Wq_g.T).T and kT likewise: [512, S] with head-dim on partitions
    -> head h occupies 64 partitions; head pairs share a 128-partition chunk
  - v  = x @ Wv_g.T natural [S, 512], stored per-head [S, 8, 64]
  - scores computed TRANSPOSED: P.T[k, q] = sum_d kT[d,k] qT[d,q]. Head pairs
    run as row-tiled concurrent matmuls (K=64 each at partition bases 0/64).
  - exp straight out of PSUM ([128, 2, 512] supertiles), strictly
    ALTERNATING ScalarE ACTIVATE / VectorE one-op Schraudolph exp2 (bf16 bit
    pattern): consecutive supertile slots free on different engines, so the
    3-deep psum-slot reuse chain never waits behind two exps on one queue.
  - pv: h.T[d, q] for a head PAIR accumulates in ONE psum bank as two
    COLUMN-TILED concurrent matmuls (M=64 at array cols 0/64 -> psum
    partitions 0-63 / 64-127).  This halves pv streaming vs M=65 serial.
  - softmax denominators from 4-way column-tiled ones-matmuls (M=32
    replicated rows at cols 0/32/64/96): series (headA,even kt), (B,even),
    (A,odd), (B,odd) accumulate in one bank (per-partition has_written
    scoping makes the four start/stop groups independent); host adds
    even+odd and divides.
  - evac: ScalarE copies psum->SBUF (h as bf16, rowsum bank f32; VectorE
    keeps the projection-bias evacuations), DMA out on alternating queues
    (sync/gpsimd); host normalizes + reassembles.

Scheduling: the Tile scheduler is greedy (per-engine ready-heaps, emission
order = priority), so the emission cadence sets the pipeline:
  - per iteration (hp, qb), batches of 2 score-supertiles; the PREVIOUS
    iteration's pv rides batches 0-5 (3,3,3,3,2,2 slots, front-loaded so the
    single pv bank is evacuated ~2 batches before reuse) and its rowsums
    fill the otherwise scores-only batches 6-7 (4+4 col-tiled slots).
  - projections stream as deadline-paced background work borrowing score-
    supertile PSUM slots; the last iteration's own pv/rs run with an
    emission lag, borrowing one sc slot (banks 0/1).
  - input DMA: 512KB pieces in first-use order; x blocks on the sync HWDGE
    queue, wq + x second-halves on gpsimd SWDGE, bias-pack/wk_h1/wv on the
    (slow) scalar HWDGE queue.  The three queues share ~250-360GB/s of HBM,
    and the first iteration consumes all 7.3MB of input, so arrival order
    is the head-latency lever.
PSUM: 6 banks of score supertiles (bufs=3) + 1 pv bank + 1 rowsum bank.

Measured on hardware (NTFF): ~299 us per core (baseline with M=65 serial pv
+ ones-column rowsums: ~314 us), relative error ~1.0e-2 vs fp32 reference.
TensorE-bound: ~217 us of matmul streaming + ~45 us of residual gaps
(~100ns weight-geometry drains at row-tiled<->full-row switches, exp-slot
latency chain, input-arrival trickle in iteration 0), plus ~15 us head
(runtime preamble + first-piece DMA) and ~6 us epilogue.  ScalarE ~192 us
and VectorE ~197 us fit underneath.
"""

import sys

sys.path.insert(0, "/opt/trn_rl_repo")

from contextlib import ExitStack

import ml_dtypes
import numpy as np

import concourse.bass as bass
import concourse.tile as tile
from concourse import bacc, mybir
from concourse.bass_utils import run_bass_kernel_spmd

F32 = mybir.dt.float32
I16 = mybir.dt.int16
BF16 = mybir.dt.bfloat16
EXP = mybir.ActivationFunctionType.Exp

B, S, D, H = 4, 2048, 1024, 16
HD = D // H          # 64
DG = D // 2          # 512 features per head-group (8 heads)
P = 128
KC = D // P          # 8 contraction chunks
NB = S // 512        # 4 token blocks of 512
NT = S // P          # 16 token tiles of 128
LH = 8               # local heads per core
# Schraudolph exp2 constants in int16/bf16: i16 = s*(2^7/(8 ln2)) +
# (127*2^7 - C); bitcasting the int16 as bf16 gives ~= exp(s/8) in one DVE
# op (|rel err| < 4.2%, ~zero mean; C tuned empirically).
EXP2_A = float((1 << 7) / (8.0 * np.log(2.0)))
EXP2_B = float(127.0 * (1 << 7) - 7.25)


def _build_attention(tc: tile.TileContext, ctx: ExitStack, io):
    nc = tc.nc
    xT, wqT, wkT, wvT, biases, out_h, out_rs = io

    const_pool = ctx.enter_context(tc.tile_pool(name="const", bufs=1))
    big_pool = ctx.enter_context(tc.tile_pool(name="big", bufs=1))
    pt_pool = ctx.enter_context(tc.tile_pool(name="ptp", bufs=36))
    st_pool = ctx.enter_context(tc.tile_pool(name="stp", bufs=3))
    # PSUM: 6 banks of score supertiles (also borrowed by projections and the
    # last iteration's in-flight accumulators) + 1 pv bank + 1 rowsum bank.
    psum_sc = ctx.enter_context(
        tc.tile_pool(name="psc", bufs=3, space=bass.MemorySpace.PSUM)
    )
    psum_pv = ctx.enter_context(
        tc.tile_pool(name="ppv", bufs=1, space=bass.MemorySpace.PSUM)
    )
    psum_rs = ctx.enter_context(
        tc.tile_pool(name="prs", bufs=1, space=bass.MemorySpace.PSUM)
    )

    # biases arrive as ONE packed [128, 264] f32 piece (cols 0-3 bq, 4-7 bk,
    # 8-263 bv broadcast as bf16 pairs) -- a single partition-major DMA; the
    # 16B-per-partition layouts cost ~45ns/descriptor if shipped separately.
    bias_sb = const_pool.tile([P, 264], F32, name="bias_sb", tag="bias")
    bq_sb = bias_sb[:, 0:4]
    bk_sb = bias_sb[:, 4:8]
    bv_sb = const_pool.tile([P, LH, HD], BF16, name="bv_sb", tag="bv")
    ones_sb = const_pool.tile([P, 32], BF16, name="ones_sb", tag="ones")
    nc.vector.memset(ones_sb[:], 1.0)

    x_sb = big_pool.tile([P, KC, S], BF16, name="x_sb", tag="x_sb")
    wq_sb = big_pool.tile([P, KC, DG], BF16, name="wq_sb", tag="wq_sb")
    wk_sb = big_pool.tile([P, KC, DG], BF16, name="wk_sb", tag="wk_sb")
    wv_sb = big_pool.tile([P, KC, DG], BF16, name="wv_sb", tag="wv_sb")

    # Input DMA: split into 512KB pieces, spread across the three DMA-trigger
    # queues (sync=HWDGE q1, scalar=HWDGE q10, gpsimd=SWDGE q0) in first-use
    # order.  h0/h1 = contraction-chunk halves (c 0-3 / 4-7).
    hc = KC // 2
    half = hc * P

    def dma_w(eng, w_sb, wT, h):
        eng.dma_start(
            w_sb[:, h * hc : (h + 1) * hc, :],
            wT[h * half : (h + 1) * half].rearrange("(c p) d -> p c d", p=P),
        )

    def dma_x(eng, h, nb):
        eng.dma_start(
            x_sb[:, h * hc : (h + 1) * hc, nb * 512 : (nb + 1) * 512],
            xT[h * half : (h + 1) * half, nb * 512 : (nb + 1) * 512].rearrange(
                "(c p) n -> p c n", p=P
            ),
        )

    # Pieces ordered by first-use deadline; the three queues share ~358GB/s
    # of HBM, so early pieces get the full bandwidth only if later pieces
    # queue behind them.
    # sync queue (fastest): x block pipeline in deadline order
    dma_x(nc.sync, 0, 0)
    dma_w(nc.sync, wk_sb, wkT, 0)
    dma_x(nc.sync, 0, 1)
    dma_x(nc.sync, 0, 2)
    dma_x(nc.sync, 0, 3)
    # gpsimd queue: wq (gates the first projection with x0), then x h1 halves
    dma_w(nc.gpsimd, wq_sb, wqT, 0)
    dma_w(nc.gpsimd, wq_sb, wqT, 1)
    dma_x(nc.gpsimd, 1, 0)
    dma_x(nc.gpsimd, 1, 1)
    dma_x(nc.gpsimd, 1, 2)
    dma_x(nc.gpsimd, 1, 3)
    # scalar queue (slowest): tiny bias pack, wk second half, v weights
    nc.scalar.dma_start(bias_sb[:], biases[:])
    dma_w(nc.scalar, wk_sb, wkT, 1)
    dma_w(nc.scalar, wv_sb, wvT, 0)
    dma_w(nc.scalar, wv_sb, wvT, 1)
    # unpack the broadcast v bias (bf16 pairs packed in the f32 piece)
    nc.vector.tensor_copy(bv_sb[:], bias_sb[:, 8:264].bitcast(BF16))

    qT_sb = big_pool.tile([P, 4, S], BF16, name="qT_sb", tag="qT_sb")
    kT_sb = big_pool.tile([P, 4, S], BF16, name="kT_sb", tag="kT_sb")
    v_sb = big_pool.tile([P, NT, LH, HD], BF16, name="v_sb", tag="v_sb")

    def proj_qk(w_sb, b_sb, dst_sb, mt, nb, pool, tag, on_scalar=False):
        # dst.T tile: rows = W-slice features (mt), cols = tokens.  The bias
        # evacuation splits between the engines (k on ScalarE, q on VectorE)
        # so neither queue carries two evacuations between its exps.
        ps = pool.tile([P, 512], F32, name="ps_qk", tag=tag)
        for c in range(KC):
            nc.tensor.matmul(
                ps[:],
                lhsT=w_sb[:, c, mt * P : (mt + 1) * P],
                rhs=x_sb[:, c, nb * 512 : (nb + 1) * 512],
                start=(c == 0),
                stop=(c == KC - 1),
            )
        dst = dst_sb[:, mt, nb * 512 : (nb + 1) * 512]
        if on_scalar:
            nc.scalar.activation(
                dst, ps[:], mybir.ActivationFunctionType.Identity,
                bias=b_sb[:, mt : mt + 1],
            )
        else:
            nc.vector.tensor_scalar_add(dst, ps[:], b_sb[:, mt : mt + 1])

    def proj_v(tt, pool, tag):
        ps_v = pool.tile([P, LH, HD], F32, name="ps_v", tag=tag)
        for c in range(KC):
            nc.tensor.matmul(
                ps_v[:],
                lhsT=x_sb[:, c, tt * P : (tt + 1) * P],
                rhs=wv_sb[:, c, :],
                start=(c == 0),
                stop=(c == KC - 1),
            )
        nc.vector.tensor_add(v_sb[:, tt, :, 0:HD], ps_v[:], bv_sb[:])

    # Upfront (borrowing idle score-supertile PSUM slots): just enough q/k
    # projection to unblock the first score matmuls.
    proj_qk(wq_sb, bq_sb, qT_sb, 0, 0, psum_sc, "sc")
    proj_qk(wk_sb, bk_sb, kT_sb, 0, 0, psum_sc, "sc")

    def proj_qk2(mt, nb, pool, tag):
        # paired k+q projection for the same (mt, nb): chunk-interleaved so
        # the PE can co-stream the two accumulation chains (they write two
        # DIFFERENT psum banks of one [P, 2, 512] tile -- same 2-bank slot a
        # score supertile uses).  Both bias evacuations stay on VectorE: a
        # k-evac on ScalarE's queue delays the exps that gate sc-slot reuse.
        ps2 = pool.tile([P, 2, 512], F32, name="ps_qk2", tag=tag)
        for c in range(KC):
            for i, w_sb in ((0, wk_sb), (1, wq_sb)):
                nc.tensor.matmul(
                    ps2[:, i, :],
                    lhsT=w_sb[:, c, mt * P : (mt + 1) * P],
                    rhs=x_sb[:, c, nb * 512 : (nb + 1) * 512],
                    start=(c == 0),
                    stop=(c == KC - 1),
                )
        nc.vector.tensor_scalar_add(
            kT_sb[:, mt, nb * 512 : (nb + 1) * 512], ps2[:, 0, :], bk_sb[:, mt : mt + 1]
        )
        nc.vector.tensor_scalar_add(
            qT_sb[:, mt, nb * 512 : (nb + 1) * 512], ps2[:, 1, :], bq_sb[:, mt : mt + 1]
        )

    def proj_v2(tt, pool, tag):
        # paired v projection for token tiles (tt, tt+1), chunk-interleaved
        ps_v2 = pool.tile([P, 2, LH, HD], F32, name="ps_v2", tag=tag)
        for c in range(KC):
            for i in range(2):
                nc.tensor.matmul(
                    ps_v2[:, i, :, :],
                    lhsT=x_sb[:, c, (tt + i) * P : (tt + i + 1) * P],
                    rhs=wv_sb[:, c, :],
                    start=(c == 0),
                    stop=(c == KC - 1),
                )
        nc.vector.tensor_add(v_sb[:, tt, :, 0:HD], ps_v2[:, 0, :, :], bv_sb[:])
        nc.vector.tensor_add(v_sb[:, tt + 1, :, 0:HD], ps_v2[:, 1, :, :], bv_sb[:])

    # Everything else is background work emitted into the attention stream by
    # deadline (in units of global score-supertiles), borrowing sc PSUM slots.
    # q and k of the same (mt, nb) are one chunk-interleaved pair at k's
    # (earlier) deadline; v tiles pair (tt, tt+1).
    NST = NT  # supertiles per (hp, qb) iteration; st j == key tile kt=j
    bg = []
    for nb in range(1, NB):
        bg.append((4 * nb + 3, lambda pool, tag, nb=nb: proj_qk2(0, nb, pool, tag)))
    for tt in range(0, NT, 2):
        # v[kt] is first read by the pv of iteration (0,0), cascaded during
        # iteration (0,1).  Spread the v tile pairs through iteration 0 (which
        # has no pv/rs work, so both PE and the elementwise engines have slack
        # there) instead of bursting all 16 at the start of iteration 1.
        bg.append((NST + tt - 4, lambda pool, tag, tt=tt: proj_v2(tt, pool, tag)))
    for mt in range(1, 4):
        for nb in range(NB):
            bg.append((4 * NST * mt + 4 * nb - 13, lambda pool, tag, mt=mt, nb=nb: proj_qk2(mt, nb, pool, tag)))
    bg.sort(key=lambda t: t[0])
    bg_pos = 0
    stg = 0

    def drain_bg(pool, tag, limit=None):
        nonlocal bg_pos
        n = 0
        while bg_pos < len(bg) and bg[bg_pos][0] <= stg + 6:
            if limit is not None and n >= limit:
                break
            bg[bg_pos][1](pool, tag)
            bg_pos += 1
            n += 1

    def emit_score(hp, qb, kt):
        # one supertile: row-tiled concurrent K=64 pair + exp -> pt (bf16)
        nonlocal stg
        ps_st = psum_sc.tile([P, 2, 512], F32, name="ps_st", tag="sc")
        pt_st = pt_pool.tile([P, 2, 512], BF16, name="pt_st", tag="pt")
        for h2 in range(2):
            base = h2 * 64
            nc.tensor.matmul(
                ps_st[:, h2, :],
                lhsT=kT_sb[base : base + 64, hp, kt * P : (kt + 1) * P],
                rhs=qT_sb[base : base + 64, hp, qb * 512 : (qb + 1) * 512],
                start=True,
                stop=True,
            )
        if stg % 2 == 1:
            # alternate exp strictly Scalar/Vector so consecutive supertile
            # slots free on different engines (the psum-slot reuse latency
            # chain never waits behind two exps on one queue): odd tiles on
            # VectorE via Schraudolph exp2 in the bf16 bit pattern
            nc.vector.tensor_scalar(
                pt_st[:].bitcast(I16),
                ps_st[:],
                EXP2_A,
                EXP2_B,
                mybir.AluOpType.mult,
                mybir.AluOpType.add,
            )
        else:
            nc.scalar.activation(pt_st[:], ps_st[:], EXP, scale=1.0 / 8.0)
        stg += 1
        return pt_st

    def emit_pv(pv_bank, hp, kt, pt_st):
        # two column-tiled concurrent matmuls: head A -> psum partitions
        # 0-63, head B -> 64-127, one bank, independent accumulation series
        for h2 in range(2):
            nc.tensor.matmul(
                pv_bank[h2 * 64 : (h2 + 1) * 64, :],
                lhsT=v_sb[:, kt, hp * 2 + h2, :],
                rhs=pt_st[:, h2, :],
                start=(kt == 0),
                stop=(kt == NT - 1),
            )

    def emit_rs(rs_bank, m, pts):
        # one 4-way column-tiled slot: rowsums of kt pair (2m, 2m+1) for both
        # heads.  Series (A,even)(B,even)(A,odd)(B,odd) accumulate in the
        # 32-partition strips at psum partitions 0/32/64/96 (M=32 replicated
        # rows so the whole bank holds valid data for a contiguous evac).
        for i in range(2):  # i: 0=even kt, 1=odd kt
            kt = 2 * m + i
            for h2 in range(2):
                r = 64 * i + 32 * h2
                nc.tensor.matmul(
                    rs_bank[r : r + 32, :],
                    lhsT=ones_sb[:],
                    rhs=pts[kt][:, h2, :],
                    start=(kt < 2),
                    stop=(kt >= NT - 2),
                    tile_position=(0, r),
                )

    def emit_evac_h(pv_bank, hp, qb, it):
        # evacuation copies run on ScalarE (its exp share drops to 1/2 under
        # the strict alternation, VectorE keeps the proj-bias evacuations)
        h_stage = st_pool.tile([P, 512], BF16, name="h_stage", tag="hst")
        nc.scalar.copy(h_stage[:], pv_bank[:])
        eng = nc.sync if it % 2 == 0 else nc.gpsimd
        eng.dma_start(out_h[hp, qb], h_stage[:])

    def emit_evac_rs(rs_bank, hp, qb, it):
        rs_stage = st_pool.tile([P, 512], F32, name="rs_stage", tag="rst")
        nc.scalar.copy(rs_stage[:], rs_bank[:])
        eng = nc.sync if it % 2 == 0 else nc.gpsimd
        eng.dma_start(out_rs[hp, qb], rs_stage[0:97:32, :])

    # Batches: scores runs of 3 (the max the 3-slot exp pipeline sustains),
    # pv in long runs of 6 (fewer row-tiled<->col-tiled weight-geometry
    # switches, each costing a ~100ns drain), pv done ~60% in so the single
    # pv bank is evacuated well before reuse; rowsum slots fill the last two
    # scores-only batches.
    S_SCHED = (2, 2, 2, 2, 2, 2, 2, 2)
    PV_SCHED = (3, 3, 3, 3, 2, 2, 0, 0)
    RS_SCHED = (0, 0, 0, 0, 0, 0, 4, 4)
    PV_BASE = np.cumsum((0,) + PV_SCHED)
    RS_BASE = np.cumsum((0,) + RS_SCHED)
    S_BASE = np.cumsum((0,) + S_SCHED)
    # lagged in-iteration pv for the last iteration: kts per batch, all
    # within the already-emitted supertile range
    PV15_SCHED = ((), (), (), (0, 1, 2), (3, 4, 5), (6, 7, 8), (9, 10, 11), (12, 13, 14))

    prev = None  # (hp, qb, [pt_st per kt])
    for it in range(16):
        hp, qb = divmod(it, 4)
        last = it == 15
        if prev is not None:
            pv_bank = psum_pv.tile([P, 512], F32, name="pv_bank", tag="pv")
            rs_bank = psum_rs.tile([P, 512], F32, name="rs_bank", tag="rs")
        if last:
            # no iteration 16 to cascade into: this iteration's own pv/rs run
            # here with an emission lag, borrowing one score-supertile slot
            # (pv in bank 0, rowsums in bank 1 at the tail).
            acc15 = psum_sc.tile([P, 2, 512], F32, name="ps_st", tag="sc")
            pv15 = acc15[:, 0, :]
        cur_pts = []
        for j in range(len(S_SCHED)):
            for kt in range(S_BASE[j], S_BASE[j + 1]):
                cur_pts.append(emit_score(hp, qb, kt))
            if prev is not None:
                for kt in range(PV_BASE[j], PV_BASE[j + 1]):
                    emit_pv(pv_bank, prev[0], kt, prev[2][kt])
                if PV_BASE[j] < NT <= PV_BASE[j + 1]:
                    emit_evac_h(pv_bank, prev[0], prev[1], it)
                for m in range(RS_BASE[j], RS_BASE[j + 1]):
                    emit_rs(rs_bank, m, prev[2])
                if RS_BASE[j] < 8 <= RS_BASE[j + 1]:
                    emit_evac_rs(rs_bank, prev[0], prev[1], it)
            if last:
                for kt in PV15_SCHED[j]:
                    emit_pv(pv15, hp, kt, cur_pts[kt])
                if j >= 6:
                    # pull half of this iteration's own rowsums off the tail
                    # (their kt pairs are long since exp'd)
                    for m in (2 * (j - 6), 2 * (j - 6) + 1):
                        emit_rs(acc15[:, 1, :], m, cur_pts)
            drain_bg(psum_sc, "sc")
        prev = (hp, qb, cur_pts)

    # tail flush of the last iteration: final pv slot, remaining rowsums
    emit_pv(pv15, 3, NT - 1, cur_pts[NT - 1])
    for m in range(4, 8):
        emit_rs(acc15[:, 1, :], m, cur_pts)
    emit_evac_h(pv15, 3, 3, 15)
    emit_evac_rs(acc15[:, 1, :], 3, 3, 16)
    assert PV_BASE[-1] == NT and RS_BASE[-1] == 8 and S_BASE[-1] == NT


def build_program():
    nc = bacc.Bacc(
        "TRN2", target_bir_lowering=False, debug=False, num_devices=8
    )
    xT = nc.dram_tensor("xT", [D, S], BF16, kind="ExternalInput").ap()
    wqT = nc.dram_tensor("wqT", [D, DG], BF16, kind="ExternalInput").ap()
    wkT = nc.dram_tensor("wkT", [D, DG], BF16, kind="ExternalInput").ap()
    wvT = nc.dram_tensor("wvT", [D, DG], BF16, kind="ExternalInput").ap()
    biases = nc.dram_tensor("biases", [P, 264], F32, kind="ExternalInput").ap()
    # h.T per (hp, qb): [2 heads x 64 dims, 512 q] bf16, plus the 4 softmax
    # rowsum rows [Aeven, Beven, Aodd, Bodd] fp32 (host adds even+odd)
    out_h = nc.dram_tensor("out_h", [4, 4, P, 512], BF16, kind="ExternalOutput").ap()
    out_rs = nc.dram_tensor("out_rs", [4, 4, 4, 512], F32, kind="ExternalOutput").ap()

    with tile.TileContext(nc) as tc, ExitStack() as ctx:
        _build_attention(tc, ctx, (xT, wqT, wkT, wvT, biases, out_h, out_rs))
    nc.compile()
    return nc


def make_in_maps(x, Wq, bq, Wk, bk, Wv, bv):
    bf = ml_dtypes.bfloat16
    x = np.asarray(x, np.float32)
    in_maps = []
    for c in range(8):
        b, g = c // 2, c % 2
        sl = slice(DG * g, DG * (g + 1))
        bq_pm = np.asarray(bq, np.float32)[sl].reshape(4, P).T      # [128, 4]
        bk_pm = np.asarray(bk, np.float32)[sl].reshape(4, P).T
        bv_bc = np.ascontiguousarray(
            np.broadcast_to(np.asarray(bv, np.float32)[sl], (P, DG))
        ).astype(bf)
        biases = np.concatenate(
            [bq_pm, bk_pm, bv_bc.view(np.float32)], axis=1
        )  # [128, 264] f32: bq | bk | bv-as-bf16-pairs
        in_maps.append(
            {
                "xT": np.ascontiguousarray(x[b].T).astype(bf),
                "wqT": np.ascontiguousarray(np.asarray(Wq, np.float32)[sl].T).astype(bf),
                "wkT": np.ascontiguousarray(np.asarray(Wk, np.float32)[sl].T).astype(bf),
                "wvT": np.ascontiguousarray(np.asarray(Wv, np.float32)[sl].T).astype(bf),
                "biases": np.ascontiguousarray(biases),
            }
        )
    return in_maps


def assemble(outs):
    res = np.empty((B, S, D), np.float32)
    for c in range(8):
        b, g = c // 2, c % 2
        oh = np.asarray(outs[c]["out_h"], np.float32)   # [4 hp, 4 qb, 128, 512]
        ors = np.asarray(outs[c]["out_rs"], np.float32)  # [4 hp, 4 qb, 4, 512]
        for hp in range(4):
            for h2 in range(2):
                lh = 2 * hp + h2
                ht = oh[hp, :, h2 * 64 : (h2 + 1) * 64, :]      # [4 qb, 64, 512]
                rs = ors[hp, :, h2, :] + ors[hp, :, 2 + h2, :]  # [4 qb, 512]
                hn = ht / rs[:, None, :]
                # -> [S, 64]
                res[b, :, DG * g + lh * 64 : DG * g + (lh + 1) * 64] = (
                    hn.transpose(0, 2, 1).reshape(S, HD)
                )
    return res


_NC_CACHE = None


def _get_program():
    global _NC_CACHE
    if _NC_CACHE is None:
        _NC_CACHE = build_program()
    return _NC_CACHE


def _install_ntff_hook():
    """The agent image lacks ``antenv.axon_hooks``; recreate it and install
    the ctypes NTFF-profiling hook against libaxon_pjrt.so (the same thing
    trn_boot does when the module exists). Only used for trace=True runs."""
    import contextlib
    import ctypes
    import types

    try:
        from antenv.axon_hooks import get_axon_ntff_profile_hook  # noqa: F401

        return
    except ImportError:
        pass

    so_path = "/opt/axon/libaxon_pjrt.so"
    lib = ctypes.CDLL(so_path)
    if not hasattr(lib, "axon_start_nrt_profile"):
        return
    lib.axon_start_nrt_profile.argtypes = [
        ctypes.POINTER(ctypes.c_int64),
        ctypes.c_size_t,
    ]
    lib.axon_start_nrt_profile.restype = ctypes.c_int64
    lib.axon_stop_nrt_profile.argtypes = [ctypes.c_char_p]
    lib.axon_stop_nrt_profile.restype = ctypes.c_int64

    @contextlib.contextmanager
    def _hook(output_dir, device_ids):
        import jax

        jax.devices()
        if device_ids:
            ids = (ctypes.c_int64 * len(device_ids))(*device_ids)
            rc = lib.axon_start_nrt_profile(ids, len(device_ids))
        else:
            rc = lib.axon_start_nrt_profile(None, 0)
        if rc != 0:
            raise RuntimeError(f"axon_start_nrt_profile rc={rc}")
        try:
            yield
        finally:
            n = lib.axon_stop_nrt_profile(str(output_dir).encode())
            print(f"ntff profile: {n} file(s) written to {output_dir}")

    mod = types.ModuleType("antenv.axon_hooks")
    mod._hook = _hook
    mod.set_axon_ntff_profile_hook = lambda h: setattr(mod, "_hook", h)
    mod.get_axon_ntff_profile_hook = lambda: mod._hook
    sys.modules["antenv.axon_hooks"] = mod

    # artifact upload reaches for a shared bucket that this container can't
    # see; the local tmpdir is all the profile pipeline needs
    import concourse.bass_utils as bu

    bu.upload_artifacts = lambda tmpdir: tmpdir


def kernel(x, Wq, bq, Wk, bk, Wv, bv, trace=False, tmpdir=None):
    nc = _get_program()
    if trace:
        _install_ntff_hook()
    in_maps = make_in_maps(x, Wq, bq, Wk, bk, Wv, bv)
    res = run_bass_kernel_spmd(
        nc, in_maps, core_ids=list(range(8)), trace=trace, tmpdir=tmpdir
    )
    full = assemble(
        [
            {"out_h": res.results[c]["out_h"], "out_rs": res.results[c]["out_rs"]}
            for c in range(8)
        ]
    )
    if trace:
        kernel.last_results = res
    return full



# revision 34
# speedup vs baseline: 1.0477x; 1.0477x over previous
"""Multi-head attention layer on 8 Trainium2 NeuronCores.

Problem: B=4, S=2048, D=1024, H=16 heads (hd=64), fp32 in/out.

Sharding: core c -> (batch b = c//2, head-group g = c%2). Each core computes
8 heads of one batch element. Fully data/tensor-parallel; no collectives.

Per-core dataflow (bf16 matmuls, fp32 PSUM accumulation):
  - host ships x[b].T (D on partitions) and W[g-slice].T, cast to bf16;
    bq/bk/bv packed into ONE [128, 264] f32 piece (tiny per-partition DMAs
    cost ~45ns/descriptor if shipped separately).
  - qT = (x @ Wq_g.T).T and kT likewise: [512, S] with head-dim on partitions
    -> head h occupies 64 partitions; head pairs share a 128-partition chunk
  - v  = x @ Wv_g.T natural [S, 512], stored per-head [S, 8, 64]
  - scores computed TRANSPOSED: P.T[k, q] = sum_d kT[d,k] qT[d,q]. Head pairs
    run as row-tiled concurrent matmuls (K=64 each at partition bases 0/64).
  - exp straight out of PSUM ([128, 2, 512] supertiles), strictly
    ALTERNATING ScalarE ACTIVATE / VectorE one-op Schraudolph exp2 (bf16 bit
    pattern): consecutive supertile slots free on different engines, so the
    3-deep psum-slot reuse chain never waits behind two exps on one queue.
  - pv: h.T[d, q] for a head PAIR accumulates in ONE psum bank as two
    COLUMN-TILED concurrent matmuls (M=64 at array cols 0/64 -> psum
    partitions 0-63 / 64-127).  This halves pv streaming vs M=65 serial.
  - softmax denominators from 4-way column-tiled ones-matmuls (M=32
    replicated rows at cols 0/32/64/96): series (headA,even kt), (B,even),
    (A,odd), (B,odd) accumulate in one bank (per-partition has_written
    scoping makes the four start/stop groups independent); host adds
    even+odd and divides.
  - evac: ScalarE copies psum->SBUF (h as bf16, rowsum bank f32; VectorE
    keeps the projection-bias evacuations), DMA out on alternating queues
    (sync/gpsimd); host normalizes + reassembles.

Scheduling: the Tile scheduler is greedy (per-engine ready-heaps, emission
order = priority), so the emission cadence sets the pipeline:
  - per iteration (hp, qb), batches of 2 score-supertiles; the PREVIOUS
    iteration's pv rides batches 0-5 (3,3,3,3,2,2 slots, front-loaded so the
    single pv bank is evacuated ~2 batches before reuse) and its rowsums
    fill the otherwise scores-only batches 6-7 (4+4 col-tiled slots).
  - projections stream as deadline-paced background work borrowing score-
    supertile PSUM slots; the last iteration's own pv/rs run with an
    emission lag, borrowing one sc slot (banks 0/1).
  - input DMA: 512KB pieces in first-use order; x blocks on the sync HWDGE
    queue, wq + x second-halves on gpsimd SWDGE, bias-pack/wk_h1/wv on the
    (slow) scalar HWDGE queue.  The three queues share ~250-360GB/s of HBM,
    and the first iteration consumes all 7.3MB of input, so arrival order
    is the head-latency lever.
PSUM: 6 banks of score supertiles (bufs=3) + 1 pv bank + 1 rowsum bank.

Measured on hardware (NTFF): ~299 us per core (baseline with M=65 serial pv
+ ones-column rowsums: ~314 us), relative error ~1.0e-2 vs fp32 reference.
NOTE the device has two clock states: boosted (~299 us, matmul dur ~379 ns
for a 512-col stream) and nominal (~355 us, ~454 ns) -- compare variants by
the matmul-duration mode, not wall time.

Optimization directions MEASURED AND REJECTED (all at boosted clock, vs
299.4 us):
  - fp8e4 (DoubleRow) anywhere: projections w/ fp8 x,W -> 6.3e-2 rel err;
    pt in e4m3 -> 4-7e-2 (normal range spans only 14 octaves; p<1 lands in
    subnormals; real data has max s/8 = 9.0, e^9=8082); fp8 v alone 2.4e-2.
    Gate is 2e-2 -> all dead.  v as fp8 hi+lo is error-free but needs fp8 pt.
  - GpSimd offload (evacs or a third exp engine): GPSIMD cannot access PSUM
    (BIR verifier hard error) -> dead.
  - pv kt-pair "quads" + chunk-interleaved q/k projection pairs: 316 us.
    Interleaved dual-bank accumulation chains do NOT co-stream (still ~216
    ns/stream) and evac routing to ScalarE stalls the exp queue.
  - proj psum moved off the sc ring to idle rs/pv banks: 300.9 us (neutral;
    ring occupancy was never the constraint).
  - score runs of 3 (S_SCHED 3,3,3,3,2,2): 319.7 us -- the 2-engine exp
    throughput (~635 ns/supertile) can't feed longer runs.
  - per-supertile split exp (ScalarE head A + VectorE head B concurrently,
    halving slot-release latency): 316 us -- doubled op count overhead wins.
  - v projections just-in-time (deadline ~NST+3+pvbatch): catastrophic
    (serializes 27 us of v matmuls into iteration 1).
  - DMA piece promotion (wk h1 / x h1nb0 earlier): neutral (head is
    bandwidth-bound: input starts ~6 us after exec, 7.3 MB lands ~31 us at
    ~286 GB/s; PE head stalls ~15 us are structural).
Group-cadence facts (NTFF): single matmul groups repeat at 216 ns, pairs
~309, rs 4-way col-tiled quads ~226 (4 co-issued streams); every entry/exit
edge of a score run costs ~100 ns extra (weight-geometry drain); score
matmuls carry ~343 ns mean semaphore wait (exp-gated psum ring).
TensorE-bound: ~217 us of matmul streaming + ~45 us of residual gaps
(~100ns weight-geometry drains at row-tiled<->full-row switches, exp-slot
latency chain, input-arrival trickle in iteration 0), plus ~15 us head
(runtime preamble + first-piece DMA) and ~6 us epilogue.  ScalarE ~192 us
and VectorE ~197 us fit underneath.
"""

import sys

sys.path.insert(0, "/opt/trn_rl_repo")

from contextlib import ExitStack

import ml_dtypes
import numpy as np

import concourse.bass as bass
import concourse.tile as tile
from concourse import bacc, mybir
from concourse.bass_utils import run_bass_kernel_spmd

F32 = mybir.dt.float32
I16 = mybir.dt.int16
BF16 = mybir.dt.bfloat16
EXP = mybir.ActivationFunctionType.Exp

B, S, D, H = 4, 2048, 1024, 16
HD = D // H          # 64
DG = D // 2          # 512 features per head-group (8 heads)
P = 128
KC = D // P          # 8 contraction chunks
NB = S // 512        # 4 token blocks of 512
NT = S // P          # 16 token tiles of 128
LH = 8               # local heads per core
# Schraudolph exp2 constants in int16/bf16: i16 = s*(2^7/(8 ln2)) +
# (127*2^7 - C); bitcasting the int16 as bf16 gives ~= exp(s/8) in one DVE
# op (|rel err| < 4.2%, ~zero mean; C tuned empirically).
EXP2_A = float((1 << 7) / (8.0 * np.log(2.0)))
EXP2_B = float(127.0 * (1 << 7) - 7.25)


def _build_attention(tc: tile.TileContext, ctx: ExitStack, io):
    nc = tc.nc
    xT, wqT, wkT, wvT, biases, out_h, out_rs = io

    const_pool = ctx.enter_context(tc.tile_pool(name="const", bufs=1))
    big_pool = ctx.enter_context(tc.tile_pool(name="big", bufs=1))
    pt_pool = ctx.enter_context(tc.tile_pool(name="ptp", bufs=36))
    st_pool = ctx.enter_context(tc.tile_pool(name="stp", bufs=3))
    # PSUM: 6 banks of score supertiles (also borrowed by projections and the
    # last iteration's in-flight accumulators) + 1 pv bank + 1 rowsum bank.
    psum_sc = ctx.enter_context(
        tc.tile_pool(name="psc", bufs=3, space=bass.MemorySpace.PSUM)
    )
    psum_pv = ctx.enter_context(
        tc.tile_pool(name="ppv", bufs=1, space=bass.MemorySpace.PSUM)
    )
    psum_rs = ctx.enter_context(
        tc.tile_pool(name="prs", bufs=1, space=bass.MemorySpace.PSUM)
    )

    # biases arrive as ONE packed [128, 264] f32 piece (cols 0-3 bq, 4-7 bk,
    # 8-263 bv broadcast as bf16 pairs) -- a single partition-major DMA; the
    # 16B-per-partition layouts cost ~45ns/descriptor if shipped separately.
    bias_sb = const_pool.tile([P, 264], F32, name="bias_sb", tag="bias")
    bq_sb = bias_sb[:, 0:4]
    bk_sb = bias_sb[:, 4:8]
    bv_sb = const_pool.tile([P, LH, HD], BF16, name="bv_sb", tag="bv")
    ones_sb = const_pool.tile([P, 32], BF16, name="ones_sb", tag="ones")
    nc.vector.memset(ones_sb[:], 1.0)

    x_sb = big_pool.tile([P, KC, S], BF16, name="x_sb", tag="x_sb")
    wq_sb = big_pool.tile([P, KC, DG], BF16, name="wq_sb", tag="wq_sb")
    wk_sb = big_pool.tile([P, KC, DG], BF16, name="wk_sb", tag="wk_sb")
    wv_sb = big_pool.tile([P, KC, DG], BF16, name="wv_sb", tag="wv_sb")

    # Input DMA: split into 512KB pieces, spread across the three DMA-trigger
    # queues (sync=HWDGE q1, scalar=HWDGE q10, gpsimd=SWDGE q0) in first-use
    # order.  h0/h1 = contraction-chunk halves (c 0-3 / 4-7).
    hc = KC // 2
    half = hc * P

    def dma_w(eng, w_sb, wT, h):
        eng.dma_start(
            w_sb[:, h * hc : (h + 1) * hc, :],
            wT[h * half : (h + 1) * half].rearrange("(c p) d -> p c d", p=P),
        )

    def dma_x(eng, h, nb):
        eng.dma_start(
            x_sb[:, h * hc : (h + 1) * hc, nb * 512 : (nb + 1) * 512],
            xT[h * half : (h + 1) * half, nb * 512 : (nb + 1) * 512].rearrange(
                "(c p) n -> p c n", p=P
            ),
        )

    # Pieces ordered by first-use deadline; the three queues share ~358GB/s
    # of HBM, so early pieces get the full bandwidth only if later pieces
    # queue behind them.
    # sync queue (fastest): x block pipeline in deadline order
    dma_x(nc.sync, 0, 0)
    dma_w(nc.sync, wk_sb, wkT, 0)
    dma_x(nc.sync, 0, 1)
    dma_x(nc.sync, 0, 2)
    dma_x(nc.sync, 0, 3)
    # gpsimd queue: wq (gates the first projection with x0), then x h1 halves
    dma_w(nc.gpsimd, wq_sb, wqT, 0)
    dma_w(nc.gpsimd, wq_sb, wqT, 1)
    dma_x(nc.gpsimd, 1, 0)
    dma_x(nc.gpsimd, 1, 1)
    dma_x(nc.gpsimd, 1, 2)
    dma_x(nc.gpsimd, 1, 3)
    # scalar queue (slowest): tiny bias pack, wk second half, v weights
    nc.scalar.dma_start(bias_sb[:], biases[:])
    dma_w(nc.scalar, wk_sb, wkT, 1)
    dma_w(nc.scalar, wv_sb, wvT, 0)
    dma_w(nc.scalar, wv_sb, wvT, 1)
    # unpack the broadcast v bias (bf16 pairs packed in the f32 piece)
    nc.vector.tensor_copy(bv_sb[:], bias_sb[:, 8:264].bitcast(BF16))

    qT_sb = big_pool.tile([P, 4, S], BF16, name="qT_sb", tag="qT_sb")
    kT_sb = big_pool.tile([P, 4, S], BF16, name="kT_sb", tag="kT_sb")
    v_sb = big_pool.tile([P, NT, LH, HD], BF16, name="v_sb", tag="v_sb")

    def proj_qk(w_sb, b_sb, dst_sb, mt, nb, pool, tag, on_scalar=False):
        # dst.T tile: rows = W-slice features (mt), cols = tokens.  The bias
        # evacuation splits between the engines (k on ScalarE, q on VectorE)
        # so neither queue carries two evacuations between its exps.
        ps = pool.tile([P, 512], F32, name="ps_qk", tag=tag)
        for c in range(KC):
            nc.tensor.matmul(
                ps[:],
                lhsT=w_sb[:, c, mt * P : (mt + 1) * P],
                rhs=x_sb[:, c, nb * 512 : (nb + 1) * 512],
                start=(c == 0),
                stop=(c == KC - 1),
            )
        dst = dst_sb[:, mt, nb * 512 : (nb + 1) * 512]
        if on_scalar:
            nc.scalar.activation(
                dst, ps[:], mybir.ActivationFunctionType.Identity,
                bias=b_sb[:, mt : mt + 1],
            )
        else:
            nc.vector.tensor_scalar_add(dst, ps[:], b_sb[:, mt : mt + 1])

    def proj_v(tt, pool, tag):
        ps_v = pool.tile([P, LH, HD], F32, name="ps_v", tag=tag)
        for c in range(KC):
            nc.tensor.matmul(
                ps_v[:],
                lhsT=x_sb[:, c, tt * P : (tt + 1) * P],
                rhs=wv_sb[:, c, :],
                start=(c == 0),
                stop=(c == KC - 1),
            )
        nc.vector.tensor_add(v_sb[:, tt, :, 0:HD], ps_v[:], bv_sb[:])

    # Upfront (borrowing idle score-supertile PSUM slots): just enough q/k
    # projection to unblock the first score matmuls.
    proj_qk(wq_sb, bq_sb, qT_sb, 0, 0, psum_sc, "sc")
    proj_qk(wk_sb, bk_sb, kT_sb, 0, 0, psum_sc, "sc")

    # Everything else is background work emitted into the attention stream by
    # deadline (in units of global score-supertiles), borrowing sc PSUM slots.
    NST = NT  # supertiles per (hp, qb) iteration; st j == key tile kt=j
    bg = []
    for nb in range(1, NB):
        bg.append((4 * nb + 3, lambda pool, tag, nb=nb: proj_qk(wk_sb, bk_sb, kT_sb, 0, nb, pool, tag)))
        bg.append((NST * nb, lambda pool, tag, nb=nb: proj_qk(wq_sb, bq_sb, qT_sb, 0, nb, pool, tag)))
    for tt in range(NT):
        # v[kt] is first read by the pv of iteration (0,0), cascaded during
        # iteration (0,1).  Spread the v tiles through iteration 0 (which has
        # no pv/rs work, so both PE and the elementwise engines have slack
        # there) instead of bursting all 16 at the start of iteration 1.
        bg.append((NST + tt - 4, lambda pool, tag, tt=tt: proj_v(tt, pool, tag)))
    for mt in range(1, 4):
        for nb in range(NB):
            bg.append((4 * NST * mt + 4 * nb - 13, lambda pool, tag, mt=mt, nb=nb: proj_qk(wk_sb, bk_sb, kT_sb, mt, nb, pool, tag)))
            bg.append((4 * NST * mt + NST * nb - 9, lambda pool, tag, mt=mt, nb=nb: proj_qk(wq_sb, bq_sb, qT_sb, mt, nb, pool, tag)))
    bg.sort(key=lambda t: t[0])
    bg_pos = 0
    stg = 0

    def drain_bg(pool, tag, limit=None):
        nonlocal bg_pos
        n = 0
        while bg_pos < len(bg) and bg[bg_pos][0] <= stg + 6:
            if limit is not None and n >= limit:
                break
            bg[bg_pos][1](pool, tag)
            bg_pos += 1
            n += 1

    def emit_score(hp, qb, kt):
        # one supertile: row-tiled concurrent K=64 pair + exp -> pt (bf16)
        nonlocal stg
        ps_st = psum_sc.tile([P, 2, 512], F32, name="ps_st", tag="sc")
        pt_st = pt_pool.tile([P, 2, 512], BF16, name="pt_st", tag="pt")
        for h2 in range(2):
            base = h2 * 64
            nc.tensor.matmul(
                ps_st[:, h2, :],
                lhsT=kT_sb[base : base + 64, hp, kt * P : (kt + 1) * P],
                rhs=qT_sb[base : base + 64, hp, qb * 512 : (qb + 1) * 512],
                start=True,
                stop=True,
            )
        if stg % 2 == 1:
            # alternate exp strictly Scalar/Vector so consecutive supertile
            # slots free on different engines (the psum-slot reuse latency
            # chain never waits behind two exps on one queue): odd tiles on
            # VectorE via Schraudolph exp2 in the bf16 bit pattern
            nc.vector.tensor_scalar(
                pt_st[:].bitcast(I16),
                ps_st[:],
                EXP2_A,
                EXP2_B,
                mybir.AluOpType.mult,
                mybir.AluOpType.add,
            )
        else:
            nc.scalar.activation(pt_st[:], ps_st[:], EXP, scale=1.0 / 8.0)
        stg += 1
        return pt_st

    def emit_pv(pv_bank, hp, kt, pt_st):
        # two column-tiled concurrent matmuls: head A -> psum partitions
        # 0-63, head B -> 64-127, one bank, independent accumulation series
        for h2 in range(2):
            nc.tensor.matmul(
                pv_bank[h2 * 64 : (h2 + 1) * 64, :],
                lhsT=v_sb[:, kt, hp * 2 + h2, :],
                rhs=pt_st[:, h2, :],
                start=(kt == 0),
                stop=(kt == NT - 1),
            )

    def emit_rs(rs_bank, m, pts):
        # one 4-way column-tiled slot: rowsums of kt pair (2m, 2m+1) for both
        # heads.  Series (A,even)(B,even)(A,odd)(B,odd) accumulate in the
        # 32-partition strips at psum partitions 0/32/64/96 (M=32 replicated
        # rows so the whole bank holds valid data for a contiguous evac).
        for i in range(2):  # i: 0=even kt, 1=odd kt
            kt = 2 * m + i
            for h2 in range(2):
                r = 64 * i + 32 * h2
                nc.tensor.matmul(
                    rs_bank[r : r + 32, :],
                    lhsT=ones_sb[:],
                    rhs=pts[kt][:, h2, :],
                    start=(kt < 2),
                    stop=(kt >= NT - 2),
                    tile_position=(0, r),
                )

    def emit_evac_h(pv_bank, hp, qb, it):
        # evacuation copies run on ScalarE (its exp share drops to 1/2 under
        # the strict alternation, VectorE keeps the proj-bias evacuations)
        h_stage = st_pool.tile([P, 512], BF16, name="h_stage", tag="hst")
        nc.scalar.copy(h_stage[:], pv_bank[:])
        eng = nc.sync if it % 2 == 0 else nc.gpsimd
        eng.dma_start(out_h[hp, qb], h_stage[:])

    def emit_evac_rs(rs_bank, hp, qb, it):
        rs_stage = st_pool.tile([P, 512], F32, name="rs_stage", tag="rst")
        nc.scalar.copy(rs_stage[:], rs_bank[:])
        eng = nc.sync if it % 2 == 0 else nc.gpsimd
        eng.dma_start(out_rs[hp, qb], rs_stage[0:97:32, :])

    # Batches: scores runs of 3 (the max the 3-slot exp pipeline sustains),
    # pv in long runs of 6 (fewer row-tiled<->col-tiled weight-geometry
    # switches, each costing a ~100ns drain), pv done ~60% in so the single
    # pv bank is evacuated well before reuse; rowsum slots fill the last two
    # scores-only batches.
    S_SCHED = (2, 2, 2, 2, 2, 2, 2, 2)
    PV_SCHED = (3, 3, 3, 3, 2, 2, 0, 0)
    RS_SCHED = (0, 0, 0, 0, 0, 0, 4, 4)
    PV_BASE = np.cumsum((0,) + PV_SCHED)
    RS_BASE = np.cumsum((0,) + RS_SCHED)
    S_BASE = np.cumsum((0,) + S_SCHED)
    # lagged in-iteration pv for the last iteration: kts per batch, all
    # within the already-emitted supertile range
    PV15_SCHED = ((), (), (), (0, 1, 2), (3, 4, 5), (6, 7, 8), (9, 10, 11), (12, 13, 14))

    prev = None  # (hp, qb, [pt_st per kt])
    for it in range(16):
        hp, qb = divmod(it, 4)
        last = it == 15
        if prev is not None:
            pv_bank = psum_pv.tile([P, 512], F32, name="pv_bank", tag="pv")
            rs_bank = psum_rs.tile([P, 512], F32, name="rs_bank", tag="rs")
        if last:
            # no iteration 16 to cascade into: this iteration's own pv/rs run
            # here with an emission lag, borrowing one score-supertile slot
            # (pv in bank 0, rowsums in bank 1 at the tail).
            acc15 = psum_sc.tile([P, 2, 512], F32, name="ps_st", tag="sc")
            pv15 = acc15[:, 0, :]
        cur_pts = []
        for j in range(len(S_SCHED)):
            for kt in range(S_BASE[j], S_BASE[j + 1]):
                cur_pts.append(emit_score(hp, qb, kt))
            if prev is not None:
                for kt in range(PV_BASE[j], PV_BASE[j + 1]):
                    emit_pv(pv_bank, prev[0], kt, prev[2][kt])
                if PV_BASE[j] < NT <= PV_BASE[j + 1]:
                    emit_evac_h(pv_bank, prev[0], prev[1], it)
                for m in range(RS_BASE[j], RS_BASE[j + 1]):
                    emit_rs(rs_bank, m, prev[2])
                if RS_BASE[j] < 8 <= RS_BASE[j + 1]:
                    emit_evac_rs(rs_bank, prev[0], prev[1], it)
            if last:
                for kt in PV15_SCHED[j]:
                    emit_pv(pv15, hp, kt, cur_pts[kt])
                if j >= 6:
                    # pull half of this iteration's own rowsums off the tail
                    # (their kt pairs are long since exp'd)
                    for m in (2 * (j - 6), 2 * (j - 6) + 1):
                        emit_rs(acc15[:, 1, :], m, cur_pts)
            drain_bg(psum_sc, "sc")
        prev = (hp, qb, cur_pts)

    # tail flush of the last iteration: final pv slot, remaining rowsums
    emit_pv(pv15, 3, NT - 1, cur_pts[NT - 1])
    for m in range(4, 8):
        emit_rs(acc15[:, 1, :], m, cur_pts)
    emit_evac_h(pv15, 3, 3, 15)
    emit_evac_rs(acc15[:, 1, :], 3, 3, 16)
    assert PV_BASE[-1] == NT and RS_BASE[-1] == 8 and S_BASE[-1] == NT


def build_program():
    nc = bacc.Bacc(
        "TRN2", target_bir_lowering=False, debug=False, num_devices=8
    )
    xT = nc.dram_tensor("xT", [D, S], BF16, kind="ExternalInput").ap()
    wqT = nc.dram_tensor("wqT", [D, DG], BF16, kind="ExternalInput").ap()
    wkT = nc.dram_tensor("wkT", [D, DG], BF16, kind="ExternalInput").ap()
    wvT = nc.dram_tensor("wvT", [D, DG], BF16, kind="ExternalInput").ap()
    biases = nc.dram_tensor("biases", [P, 264], F32, kind="ExternalInput").ap()
    # h.T per (hp, qb): [2 heads x 64 dims, 512 q] bf16, plus the 4 softmax
    # rowsum rows [Aeven, Beven, Aodd, Bodd] fp32 (host adds even+odd)
    out_h = nc.dram_tensor("out_h", [4, 4, P, 512], BF16, kind="ExternalOutput").ap()
    out_rs = nc.dram_tensor("out_rs", [4, 4, 4, 512], F32, kind="ExternalOutput").ap()

    with tile.TileContext(nc) as tc, ExitStack() as ctx:
        _build_attention(tc, ctx, (xT, wqT, wkT, wvT, biases, out_h, out_rs))
    nc.compile()
    return nc


def make_in_maps(x, Wq, bq, Wk, bk, Wv, bv):
    bf = ml_dtypes.bfloat16
    x = np.asarray(x, np.float32)
    in_maps = []
    for c in range(8):
        b, g = c // 2, c % 2
        sl = slice(DG * g, DG * (g + 1))
        bq_pm = np.asarray(bq, np.float32)[sl].reshape(4, P).T      # [128, 4]
        bk_pm = np.asarray(bk, np.float32)[sl].reshape(4, P).T
        bv_bc = np.ascontiguousarray(
            np.broadcast_to(np.asarray(bv, np.float32)[sl], (P, DG))
        ).astype(bf)
        biases = np.concatenate(
            [bq_pm, bk_pm, bv_bc.view(np.float32)], axis=1
        )  # [128, 264] f32: bq | bk | bv-as-bf16-pairs
        in_maps.append(
            {
                "xT": np.ascontiguousarray(x[b].T).astype(bf),
                "wqT": np.ascontiguousarray(np.asarray(Wq, np.float32)[sl].T).astype(bf),
                "wkT": np.ascontiguousarray(np.asarray(Wk, np.float32)[sl].T).astype(bf),
                "wvT": np.ascontiguousarray(np.asarray(Wv, np.float32)[sl].T).astype(bf),
                "biases": np.ascontiguousarray(biases),
            }
        )
    return in_maps


def assemble(outs):
    res = np.empty((B, S, D), np.float32)
    for c in range(8):
        b, g = c // 2, c % 2
        oh = np.asarray(outs[c]["out_h"], np.float32)   # [4 hp, 4 qb, 128, 512]
        ors = np.asarray(outs[c]["out_rs"], np.float32)  # [4 hp, 4 qb, 4, 512]
        for hp in range(4):
            for h2 in range(2):
                lh = 2 * hp + h2
                ht = oh[hp, :, h2 * 64 : (h2 + 1) * 64, :]      # [4 qb, 64, 512]
                rs = ors[hp, :, h2, :] + ors[hp, :, 2 + h2, :]  # [4 qb, 512]
                hn = ht / rs[:, None, :]
                # -> [S, 64]
                res[b, :, DG * g + lh * 64 : DG * g + (lh + 1) * 64] = (
                    hn.transpose(0, 2, 1).reshape(S, HD)
                )
    return res


_NC_CACHE = None


def _get_program():
    global _NC_CACHE
    if _NC_CACHE is None:
        _NC_CACHE = build_program()
    return _NC_CACHE


def _install_ntff_hook():
    """The agent image lacks ``antenv.axon_hooks``; recreate it and install
    the ctypes NTFF-profiling hook against libaxon_pjrt.so (the same thing
    trn_boot does when the module exists). Only used for trace=True runs."""
    import contextlib
    import ctypes
    import types

    try:
        from antenv.axon_hooks import get_axon_ntff_profile_hook  # noqa: F401

        return
    except ImportError:
        pass

    so_path = "/opt/axon/libaxon_pjrt.so"
    lib = ctypes.CDLL(so_path)
    if not hasattr(lib, "axon_start_nrt_profile"):
        return
    lib.axon_start_nrt_profile.argtypes = [
        ctypes.POINTER(ctypes.c_int64),
        ctypes.c_size_t,
    ]
    lib.axon_start_nrt_profile.restype = ctypes.c_int64
    lib.axon_stop_nrt_profile.argtypes = [ctypes.c_char_p]
    lib.axon_stop_nrt_profile.restype = ctypes.c_int64

    @contextlib.contextmanager
    def _hook(output_dir, device_ids):
        import jax

        jax.devices()
        if device_ids:
            ids = (ctypes.c_int64 * len(device_ids))(*device_ids)
            rc = lib.axon_start_nrt_profile(ids, len(device_ids))
        else:
            rc = lib.axon_start_nrt_profile(None, 0)
        if rc != 0:
            raise RuntimeError(f"axon_start_nrt_profile rc={rc}")
        try:
            yield
        finally:
            n = lib.axon_stop_nrt_profile(str(output_dir).encode())
            print(f"ntff profile: {n} file(s) written to {output_dir}")

    mod = types.ModuleType("antenv.axon_hooks")
    mod._hook = _hook
    mod.set_axon_ntff_profile_hook = lambda h: setattr(mod, "_hook", h)
    mod.get_axon_ntff_profile_hook = lambda: mod._hook
    sys.modules["antenv.axon_hooks"] = mod

    # artifact upload reaches for a shared bucket that this container can't
    # see; the local tmpdir is all the profile pipeline needs
    import concourse.bass_utils as bu

    bu.upload_artifacts = lambda tmpdir: tmpdir


def kernel(x, Wq, bq, Wk, bk, Wv, bv, trace=False, tmpdir=None):
    nc = _get_program()
    if trace:
        _install_ntff_hook()
    in_maps = make_in_maps(x, Wq, bq, Wk, bk, Wv, bv)
    res = run_bass_kernel_spmd(
        nc, in_maps, core_ids=list(range(8)), trace=trace, tmpdir=tmpdir
    )
    full = assemble(
        [
            {"out_h": res.results[c]["out_h"], "out_rs": res.results[c]["out_rs"]}
            for c in range(8)
        ]
    )
    if trace:
        kernel.last_results = res
    return full

